# revision 17
# baseline (speedup 1.0000x reference)
"""2-layer GAT on 8 Trainium2 NeuronCores (Bass/Tile, SPMD via axon PJRT).

Strategy (dst-sharded message passing, 3 launches, no collectives):
  A: per-core feature transform of its node shard: h~ = x @ (W1 R) and
     alpha_dst = x @ (W1 A1d). R is a per-head invertible rotation whose
     first column is a1_src, so alpha_src of a gathered row is just its
     strided column 16h -- no separate alpha table gather needed. h~ is
     stored fp16 (halves gather traffic in B).
  B: layer-1 message passing. Edge slots laid out node-major per 128-node
     destination tile (slot j of node p = chunk j, partition p), so the
     PSUM-accumulating matmul uses a constant identity lhsT (fp16: 1 cyc/
     row vs fp32's 4). Slot streams are grouped per PAIR of tiles and per
     half-table so each (pair, half) is ONE dma_gather. All per-edge
     elementwise math is batched per tile (one DVE/ACT instruction over
     all J chunks) instead of per chunk: e = g[:,0:128:16] + a_d; Lrelu;
     Exp -> M[:,:,0:8] (denominator cols, fp16); M[:,:,8:136] = g * ex
     (fp16). J matmuls accumulate denom+agg into PSUM. Finalize: divide,
     un-rotate (PE transpose + matmul by R^-1), +b1, ReLU, then fused
     layer-2 transform h2~ = relu(h1) @ [W2 | W2 a2s | W2 a2d] -> DRAM.
  C: layer-2 message passing over the same slot structure (row =
     [h2(40) | alpha2_src | pad] fp32, 256B), same batched-elementwise
     scheme, finalize with divide, +b2, log_softmax.
Host does only sharding glue: edge partitioning/sorting, half balancing,
permutations, table assembly between launches, constants.
"""
import sys
sys.path.insert(0, "/opt/trn_rl_repo")

import numpy as np
import jax

import concourse.bass as bass
import concourse.tile as tile
import concourse.mybir as mybir
from concourse import bacc
from concourse.bass2jax import _bass_exec_p, partition_id_tensor, install_neuronx_cc_hook
from jax.sharding import Mesh, PartitionSpec
from jax.experimental.shard_map import shard_map

F32 = mybir.dt.float32
F16 = mybir.dt.float16
I16 = mybir.dt.int16
AF = mybir.ActivationFunctionType
ALU = mybir.AluOpType

NEG_SLOPE = 0.2
DUMMY_ALPHA = -30000.0
P = 128


# ----------------------------------------------------------------------------
# configuration (sizes hardcoded for the graded problem; small configs used by
# the self-test harness pass explicit cfg)
# ----------------------------------------------------------------------------
class Cfg:
    def __init__(self, N, E, in_c=128, hid=16, heads=8, out_c=40, ncores=8):
        self.N, self.E = N, E
        self.in_c, self.hid, self.heads, self.out_c = in_c, hid, heads, out_c
        self.ncores = ncores
        self.npc = N // ncores                      # real nodes per core
        self.ntiles = -(-self.npc // P)             # dst tiles per core
        self.npad = self.ntiles * P                 # padded nodes per core
        # source table halves: node table slot range, dummy at local HALF
        tot = N
        self.half = -(-tot // 2)
        self.half = ((self.half + P - 1) // P) * P  # round half size up
        assert self.half + 1 <= 32767, "int16 gather index limit"
        self.c1 = heads * hid                       # layer-1 out channels (128)
        self.row2 = 64                              # layer-2 table row elems


CFG = Cfg(N=50000, E=800000)


def tile_pairs(ntiles):
    return [tuple(t for t in (g, g + 1) if t < ntiles)
            for g in range(0, ntiles, 2)]


# ----------------------------------------------------------------------------
# host-side math constants
# ----------------------------------------------------------------------------
def householder_rot(a):
    """R [k,k] with R[:,0] = a exactly, other columns orthonormal; plus R^-1."""
    k = a.shape[0]
    a = a.astype(np.float64)
    s = np.linalg.norm(a)
    if s < 1e-30:
        R = np.eye(k)
        R[0, 0] = 1.0
        return R, np.linalg.inv(R)
    u = a / s
    if u[0] > 1.0 - 1e-12:
        H = np.eye(k)
    else:
        v = u - np.eye(k)[:, 0]
        H = np.eye(k) - 2.0 * np.outer(v, v) / (v @ v)
    R = H @ np.diag([s] + [1.0] * (k - 1))
    Rinv = np.diag([1.0 / s] + [1.0] * (k - 1)) @ H
    return R, Rinv


def make_consts(cfg, W1, a1_src, a1_dst, W2, a2_src, a2_dst):
    """W1ext [in_c, c1+heads], Rinv_bd [c1, c1], W2ext [c1, out_c+2]."""
    H, D = cfg.heads, cfg.hid
    Rbd = np.zeros((cfg.c1, cfg.c1))
    Rinv = np.zeros((cfg.c1, cfg.c1))
    for h in range(H):
        R_h, Rinv_h = householder_rot(a1_src[h].astype(np.float64))
        Rbd[h * D:(h + 1) * D, h * D:(h + 1) * D] = R_h
        Rinv[h * D:(h + 1) * D, h * D:(h + 1) * D] = Rinv_h
    A1d = np.zeros((cfg.c1, H))
    for h in range(H):
        A1d[h * D:(h + 1) * D, h] = a1_dst[h].astype(np.float64)
    W1e = np.concatenate([W1.astype(np.float64) @ Rbd,
                          W1.astype(np.float64) @ A1d], axis=1)
    W2e = np.concatenate([W2.astype(np.float64),
                          W2.astype(np.float64) @ a2_src[0].astype(np.float64)[:, None],
                          W2.astype(np.float64) @ a2_dst[0].astype(np.float64)[:, None]],
                         axis=1)
    return (W1e.astype(np.float32), Rinv.astype(np.float32),
            W2e.astype(np.float32))


# ----------------------------------------------------------------------------
# host-side graph preprocessing
# ----------------------------------------------------------------------------
def balance_halves(cfg, src, dst, cap=None):
    """Assign each node a half bit, balancing each dst's in-edges between
    halves. Chunked greedy over sources (vectorized)."""
    N = cfg.N
    order = np.argsort(src, kind="stable")
    ssrc = src[order]
    sdst = dst[order]
    starts = np.searchsorted(ssrc, np.arange(N + 1))
    imb = np.zeros(N, dtype=np.int64)
    halfbit = np.zeros(N, dtype=np.int8)
    if cap is None:
        cap = cfg.half
    cnt = [0, 0]
    K = 512
    for lo in range(0, N, K):
        hi = min(lo + K, N)
        a, b = starts[lo], starts[hi]
        if b > a:
            ridx = np.minimum(starts[lo:hi] - a, b - a - 1)
            seg = np.add.reduceat(imb[sdst[a:b]], ridx)
            empty = (starts[lo + 1:hi + 1] - starts[lo:hi]) == 0
            seg = np.where(empty, 0, seg)
        else:
            seg = np.zeros(hi - lo, dtype=np.int64)
        h = (seg > 0).astype(np.int8)
        halfbit[lo:hi] = h
        cnt[0] += int((h == 0).sum())
        cnt[1] += int((h == 1).sum())
        hb_e = halfbit[ssrc[a:b]]
        np.add.at(imb, sdst[a:b], 1 - 2 * hb_e.astype(np.int64))
    # refinement sweeps (chunked, approximate)
    for _ in range(2):
        for lo in range(0, N, K):
            hi = min(lo + K, N)
            a, b = starts[lo], starts[hi]
            if b <= a:
                continue
            ridx = np.minimum(starts[lo:hi] - a, b - a - 1)
            seg = np.add.reduceat(imb[sdst[a:b]], ridx)
            empty = (starts[lo + 1:hi + 1] - starts[lo:hi]) == 0
            seg = np.where(empty, 0, seg)
            degs = (starts[lo + 1:hi + 1] - starts[lo:hi]).astype(np.int64)
            hc = halfbit[lo:hi].astype(np.int64)
            bias_excl = seg - degs * (1 - 2 * hc)
            hn = (bias_excl > 0).astype(np.int8)
            changed = hn != halfbit[lo:hi]
            if not changed.any():
                continue
            cnt[0] += int((hn[changed] == 0).sum()) - int((halfbit[lo:hi][changed] == 0).sum())
            cnt[1] += int((hn[changed] == 1).sum()) - int((halfbit[lo:hi][changed] == 1).sum())
            halfbit[lo:hi] = hn
            hb_e_new = halfbit[ssrc[a:b]].astype(np.int64)
            # recompute imb contribution for edges in this block
            d = sdst[a:b]
            hb_e_old = np.repeat(hc, degs)
            np.add.at(imb, d, (1 - 2 * hb_e_new) - (1 - 2 * hb_e_old))
    # enforce capacity
    for hh in (0, 1):
        over = cnt[hh] - cap
        if over > 0:
            ids = np.flatnonzero(halfbit == hh)[:over]
            halfbit[ids] = 1 - hh
            cnt[hh] -= over
            cnt[1 - hh] += over
    return halfbit


def preprocess(cfg, edge_index):
    """Build all per-core edge-slot structures. Returns dict.

    High-out-degree sources are REPLICATED into both half tables (spare
    int16-index capacity), making their edges' half assignment free; the
    free edges are then used to split every destination's in-edges near
    ceil/floor between the two halves, minimizing per-tile max-degree
    padding."""
    N, E, C = cfg.N, cfg.E, cfg.ncores
    loop = np.arange(N, dtype=np.int64)
    src = np.concatenate([edge_index[0], loop]).astype(np.int64)
    dst = np.concatenate([edge_index[1], loop]).astype(np.int64)

    # --- choose replicated sources (top out-degree) ---
    TABMAX = 32767  # int16 gather-index limit (incl. dummy row)
    R = min(N // 3, max(0, 2 * (TABMAX - 1) - N - 2))
    outdeg = np.bincount(src, minlength=N)
    rep = np.zeros(N, bool)
    if R > 0:
        rep[np.argsort(-outdeg, kind="stable")[:R]] = True
    nonrep = ~rep
    # halfbit for non-replicated sources (balance the fixed part)
    em = nonrep[src]
    nr_cap = TABMAX - 1 - R
    halfbit = balance_halves(cfg, src[em], dst[em], cap=nr_cap)
    halfbit[rep] = 0  # unused for replicated
    # enforce per-half capacity over non-replicated nodes only
    for hh in (0, 1):
        ids = np.flatnonzero(nonrep & (halfbit == hh))
        over = len(ids) - nr_cap
        if over > 0:
            halfbit[ids[:over]] = 1 - hh

    # --- table slots: nonrep nodes first (id order), then replicated ---
    sloc0 = np.full(N, -1, np.int64)
    sloc1 = np.full(N, -1, np.int64)
    m0 = nonrep & (halfbit == 0)
    m1 = nonrep & (halfbit == 1)
    sloc0[m0] = np.arange(int(m0.sum()))
    sloc1[m1] = np.arange(int(m1.sum()))
    repn = np.flatnonzero(rep)
    sloc0[repn] = int(m0.sum()) + np.arange(len(repn))
    sloc1[repn] = int(m1.sum()) + np.arange(len(repn))
    rows0 = int(m0.sum()) + len(repn)
    rows1 = int(m1.sum()) + len(repn)
    tabrows = max(rows0, rows1) + 1
    assert tabrows <= TABMAX + 1
    dummy = tabrows - 1

    # snake-deal sharding: global degree-desc order, node i -> core i%C,
    # rank i//C. gids[c][r] = global node id at (core c, rank r).
    degg = np.bincount(dst, minlength=N)
    gorder = np.argsort(-degg, kind="stable")
    node2core = np.empty(N, np.int32)
    node2rank = np.empty(N, np.int32)
    node2core[gorder] = (np.arange(N) % C).astype(np.int32)
    node2rank[gorder] = (np.arange(N) // C).astype(np.int32)
    gids = []
    for c in range(C):
        g = np.full(cfg.npad, -1, np.int64)
        sel = gorder[c::C]
        g[:len(sel)] = sel
        gids.append(g)

    # --- per-edge half: fixed for nonrep sources, balanced fill for rep ---
    order_d = np.argsort(dst, kind="stable")
    so, do = src[order_d], dst[order_d]
    isfree = rep[so]
    fhalf = np.where(isfree, 0, halfbit[so]).astype(np.int64)
    starts_d = np.searchsorted(do, np.arange(N + 1))
    degd = np.diff(starts_d)
    ne = len(so)
    fixed0ct = np.zeros(N, np.int64)
    fixed1ct = np.zeros(N, np.int64)
    np.add.at(fixed0ct, do[~isfree], 1 - fhalf[~isfree])
    np.add.at(fixed1ct, do[~isfree], fhalf[~isfree])
    f = degd - fixed0ct - fixed1ct
    h0t = np.clip((degd + 1) // 2, fixed0ct, degd - fixed1ct)
    free0 = h0t - fixed0ct
    gs = np.repeat(starts_d[:-1], degd)
    base = np.cumsum(isfree) - isfree
    jfree = base - base[gs]
    ehalf_o = np.where(isfree, (jfree >= free0[do]).astype(np.int64), fhalf)
    ehalf = np.empty(ne, np.int64)
    ehalf[order_d] = ehalf_o
    esloc = np.where(ehalf == 0, sloc0[src], sloc1[src])

    deg_hc = np.zeros((C, cfg.npad, 2), dtype=np.int32)
    np.add.at(deg_hc, (node2core[dst], node2rank[dst], ehalf), 1)

    # common per-tile deltas across cores (same program on all cores)
    d0t = deg_hc[:, :, 0].reshape(C, cfg.ntiles, P).max(axis=(0, 2)).astype(np.int32)
    d1t = deg_hc[:, :, 1].reshape(C, cfg.ntiles, P).max(axis=(0, 2)).astype(np.int32)
    stot = int(128 * (d0t.sum() + d1t.sum()))
    stot16 = ((stot + 15) // 16) * 16

    # slot base position of each (tile, half): pair-grouped stream order
    # [pair][half][tile-in-pair] so each (pair, half) is one gather.
    dts = (d0t, d1t)
    bases = np.zeros((cfg.ntiles, 2), dtype=np.int64)
    block_rank = np.zeros((cfg.ntiles, 2), dtype=np.int64)
    pos = 0
    br = 0
    for pair in tile_pairs(cfg.ntiles):
        for h in (0, 1):
            for t in pair:
                bases[t, h] = pos
                block_rank[t, h] = br
                pos += 128 * int(dts[h][t])
                br += 1
    assert pos == stot

    idx_flats = []
    for c in range(C):
        m = node2core[dst] == c
        hb = ehalf[m]
        vs = esloc[m]
        r = node2rank[dst[m]].astype(np.int64)
        t = r // P
        part = r % P
        key = block_rank[t, hb] * cfg.npad + r
        order = np.argsort(key, kind="stable")
        ks = key[order]
        # occurrence j of each edge within its (node, half) group
        starts_ = np.flatnonzero(np.concatenate([[1], ks[1:] != ks[:-1]]))
        gstart = np.repeat(starts_,
                           np.diff(np.concatenate([starts_, [len(ks)]])))
        j = np.arange(len(ks)) - gstart
        pos_ = (bases[t[order], hb[order]] + j * 128 + part[order])
        idx_flat = np.full(stot16, dummy, dtype=np.int16)
        idx_flat[pos_] = vs[order].astype(np.int16)
        idx_flats.append(idx_flat)

    # wrap to [128, stot16//16] with 8x replication
    idxws = []
    for c in range(C):
        w = np.zeros((P, stot16 // 16), dtype=np.int16)
        i = np.arange(stot16)
        w[i % 16, i // 16] = idx_flats[c]
        for r_ in range(1, 8):
            w[r_ * 16:(r_ + 1) * 16] = w[:16]
        idxws.append(w)

    return dict(src=src, dst=dst, sloc0=sloc0, sloc1=sloc1, gids=gids,
                tabrows=tabrows, dummy=dummy,
                d0t=d0t, d1t=d1t, bases=bases, stot16=stot16, idxws=idxws)


# ----------------------------------------------------------------------------
# SPMD runner (cached jit, modeled on bass2jax.run_bass_via_pjrt)
# ----------------------------------------------------------------------------
class SpmdRunner:
    def __init__(self, nc, n_cores):
        install_neuronx_cc_hook()
        self.nc, self.n_cores = nc, n_cores
        pname = nc.partition_id_tensor.name if nc.partition_id_tensor else None
        in_names, out_names, out_avals, zero_outs = [], [], [], []
        for alloc in nc.m.functions[0].allocations:
            if not isinstance(alloc, mybir.MemoryLocationSet):
                continue
            name = alloc.memorylocations[0].name
            if alloc.kind == "ExternalInput":
                if name != pname:
                    in_names.append(name)
            elif alloc.kind == "ExternalOutput":
                out_names.append(name)
                shape = tuple(alloc.tensor_shape)
                dtype = mybir.dt.np(alloc.dtype)
                out_avals.append(jax.core.ShapedArray(shape, dtype))
                zero_outs.append(np.zeros(shape, dtype))
        self.n_params, self.in_names, self.out_names = len(in_names), in_names, out_names
        self.zero_outs = zero_outs
        all_in = in_names + out_names + ([pname] if pname else [])

        def _body(*args):
            ops = list(args)
            if pname is not None:
                ops.append(partition_id_tensor())
            return tuple(_bass_exec_p.bind(
                *ops, out_avals=tuple(out_avals), in_names=tuple(all_in),
                out_names=tuple(out_names), lowering_input_output_aliases=(),
                sim_require_finite=False, sim_require_nnan=False, nc=nc))

        donate = tuple(range(self.n_params, self.n_params + len(out_names)))
        devices = jax.devices()[:n_cores]
        mesh = Mesh(np.asarray(devices), ("core",))
        ispec = (PartitionSpec("core"),) * (self.n_params + len(out_names))
        ospec = (PartitionSpec("core"),) * len(out_names)
        self.fn = jax.jit(shard_map(_body, mesh=mesh, in_specs=ispec,
                                    out_specs=ospec, check_rep=False),
                          donate_argnums=donate, keep_unused=True)

    def put_inputs(self, in_maps):
        concat = [np.concatenate([np.asarray(m[n]) for m in in_maps], axis=0)
                  for n in self.in_names]
        return [jax.device_put(x) for x in concat]

    def run(self, dev_inputs, retries=2):
        import time as _time
        for att in range(retries + 1):
            try:
                zeros = [np.concatenate([z] * self.n_cores, axis=0)
                         for z in self.zero_outs]
                outs = self.fn(*dev_inputs, *zeros)
                jax.block_until_ready(outs)
                return outs
            except Exception:
                if att == retries:
                    raise
                _time.sleep(60)

    def results(self, outs):
        res = [dict() for _ in range(self.n_cores)]
        for i, name in enumerate(self.out_names):
            for c, part in enumerate(np.split(np.asarray(outs[i]), self.n_cores)):
                res[c][name] = part
        return res


# ----------------------------------------------------------------------------
# launch A: h~ = x @ W1ext (per-core shard, pi-order); hrows in fp16
# ----------------------------------------------------------------------------
def build_launchA(cfg, rep=1):
    nc = bacc.Bacc("TRN2", target_bir_lowering=False, debug=False,
                   num_devices=cfg.ncores)
    w = cfg.c1 + cfg.heads
    H = cfg.heads
    xT = nc.dram_tensor("xT", [cfg.in_c, cfg.npad], F32, kind="ExternalInput")
    W1e = nc.dram_tensor("W1e", [cfg.in_c, w], F32, kind="ExternalInput")
    hrows = nc.dram_tensor("hrows", [P, cfg.ntiles * cfg.c1], F16,
                           kind="ExternalOutput")
    adrows = nc.dram_tensor("adrows", [P, cfg.ntiles * cfg.heads], F32,
                            kind="ExternalOutput")
    GRP = 8
    with tile.TileContext(nc) as tc:
        with tc.tile_pool(name="fix", bufs=1) as fix, \
             tc.tile_pool(name="xb", bufs=2) as xb, \
             tc.tile_pool(name="ob", bufs=2) as ob, \
             tc.tile_pool(name="ps", bufs=4, space="PSUM") as ps:
            wt = fix.tile([cfg.in_c, w], F32)
            nc.sync.dma_start(out=wt[:], in_=W1e[:, :])
            for _ in range(rep):
              for g in range(0, cfg.ntiles, GRP):
                  ng = min(GRP, cfg.ntiles - g)
                  xbig = xb.tile([cfg.in_c, ng * P], F32, tag="x")
                  nc.sync.dma_start(out=xbig[:],
                                    in_=xT[:, g * P:(g + ng) * P])
                  hbig = ob.tile([P, ng * cfg.c1], F16, tag="h")
                  abig = ob.tile([P, ng * H], F32, tag="a")
                  for i in range(ng):
                      pt = ps.tile([P, w], F32, tag="ps")
                      nc.tensor.matmul(pt[:], lhsT=xbig[:, i * P:(i + 1) * P],
                                       rhs=wt[:], start=True, stop=True)
                      nc.vector.tensor_copy(hbig[:, i * cfg.c1:(i + 1) * cfg.c1],
                                            pt[:, 0:cfg.c1])
                      nc.vector.tensor_copy(abig[:, i * H:(i + 1) * H],
                                            pt[:, cfg.c1:])
                  nc.sync.dma_start(
                      out=hrows[:, g * cfg.c1:(g + ng) * cfg.c1], in_=hbig[:])
                  nc.sync.dma_start(
                      out=adrows[:, g * H:(g + ng) * H], in_=abig[:])
    nc.compile()
    return nc


# ----------------------------------------------------------------------------
# launch B: layer-1 message passing + fused layer-2 feature transform
# ----------------------------------------------------------------------------
def build_launchB(cfg, d0t, d1t, stot16, nhalf, rep=1):
    H = cfg.heads
    c1 = cfg.c1
    hid = cfg.hid
    wm = H + c1          # M columns: [ex(H) | g*ex(c1)]
    nc = bacc.Bacc("TRN2", target_bir_lowering=False, debug=False,
                   num_devices=cfg.ncores, num_swdge_queues=4)
    tb0 = nc.dram_tensor("tb0", [nhalf, c1], F16, kind="ExternalInput")
    tb1 = nc.dram_tensor("tb1", [nhalf, c1], F16, kind="ExternalInput")
    idxs = nc.dram_tensor("idxs", [P, stot16 // 16], I16, kind="ExternalInput")
    adsw = nc.dram_tensor("adsw", [P, cfg.ntiles * H], F32, kind="ExternalInput")
    ident = nc.dram_tensor("ident", [P, P], F32, kind="ExternalInput")
    id16 = nc.dram_tensor("id16", [P, P], F16, kind="ExternalInput")
    rinv = nc.dram_tensor("rinv", [c1, c1], F32, kind="ExternalInput")
    w2e = nc.dram_tensor("w2e", [c1, cfg.out_c + 2], F32, kind="ExternalInput")
    b1c = nc.dram_tensor("b1c", [c1, 1], F32, kind="ExternalInput")
    h2rows = nc.dram_tensor("h2rows", [P, cfg.ntiles * cfg.row2], F32,
                            kind="ExternalOutput")

    dts = (d0t, d1t)
    pairs = tile_pairs(cfg.ntiles)
    with tile.TileContext(nc) as tc:
        with tc.tile_pool(name="fix", bufs=1) as fix, \
             tc.tile_pool(name="gp", bufs=6) as gp, \
             tc.tile_pool(name="mp", bufs=4) as mp, \
             tc.tile_pool(name="ep", bufs=6) as ep, \
             tc.tile_pool(name="sm", bufs=8) as smp, \
             tc.tile_pool(name="fin", bufs=3) as fin, \
             tc.tile_pool(name="ps", bufs=2, space="PSUM") as ps, \
             tc.tile_pool(name="ps2", bufs=2, space="PSUM") as ps2, \
             tc.tile_pool(name="ps3", bufs=2, space="PSUM") as ps3, \
             tc.tile_pool(name="ps4", bufs=2, space="PSUM") as ps4:
            it = fix.tile([P, stot16 // 16], I16)
            nc.sync.dma_start(out=it[:], in_=idxs[:, :])
            ad = fix.tile([P, cfg.ntiles * H], F32)
            nc.sync.dma_start(out=ad[:], in_=adsw[:, :])
            idt = fix.tile([P, P], F32)
            nc.sync.dma_start(out=idt[:], in_=ident[:, :])
            id16t = fix.tile([P, P], F16)
            nc.sync.dma_start(out=id16t[:], in_=id16[:, :])
            riv = fix.tile([c1, c1], F32)
            nc.sync.dma_start(out=riv[:], in_=rinv[:, :])
            w2t = fix.tile([c1, cfg.out_c + 2], F32)
            nc.sync.dma_start(out=w2t[:], in_=w2e[:, :])
            b1t = fix.tile([c1, 1], F32)
            nc.sync.dma_start(out=b1t[:], in_=b1c[:, :])

            qi = [0]
            for _ in range(rep):
                pos = 0
                for pair in pairs:
                    # ---- gather: one per half for the whole pair ----
                    gts = {}
                    segstart = {}
                    for h in (0, 1):
                        tbl = tb0 if h == 0 else tb1
                        ds = [int(dts[h][t]) for t in pair]
                        dtot = sum(ds)
                        segstart[h] = [sum(ds[:i]) for i in range(len(pair))]
                        if dtot == 0:
                            gts[h] = None
                            continue
                        gt = gp.tile([P, dtot * c1], F16, tag=f"g{h}")
                        nsp = 3 if dtot >= 3 else dtot
                        bnds = [dtot * i // nsp for i in range(nsp + 1)]
                        for si in range(nsp):
                            a, b = bnds[si], bnds[si + 1]
                            if b == a:
                                continue
                            nc.gpsimd.dma_gather(
                                out_ap=gt[:, a * c1:b * c1]
                                    .rearrange("p (c e) -> p c e", e=c1),
                                in_ap=tbl[:, :],
                                idxs_ap=it[:, (pos + a * P) // 16:
                                           (pos + b * P) // 16],
                                num_idxs=(b - a) * P,
                                num_idxs_reg=(b - a) * P,
                                elem_size=c1,
                                single_packet=False,
                                queue_num=qi[0] % 4,
                            )
                            qi[0] += 1
                        pos += dtot * P
                        gts[h] = gt
                    # ---- per tile batched compute ----
                    npair = len(pair)
                    h2big = fin.tile([P, npair * cfg.row2], F32, tag="h2")
                    for ti, t in enumerate(pair):
                        J = int(d0t[t] + d1t[t])
                        segs = []
                        for h in (0, 1):
                            dlt = int(dts[h][t])
                            if dlt:
                                segs.append((gts[h], segstart[h][ti], dlt))
                        m = mp.tile([P, J * wm], F16, tag="m")
                        mv = m[:].rearrange("p (j w) -> p j w", w=wm)
                        e = ep.tile([P, J * H], F32, tag="e")
                        e2 = ep.tile([P, J * H], F32, tag="e2")
                        adt = ad[:, t * H:(t + 1) * H] \
                            .rearrange("p (o h) -> p o h", o=1)
                        co = 0
                        for gt, s0, dlt in segs:
                            gseg = gt[:, s0 * c1:(s0 + dlt) * c1]
                            nc.vector.tensor_tensor(
                                out=e[:, co * H:(co + dlt) * H]
                                    .rearrange("p (j h) -> p j h", h=H),
                                in0=gseg.rearrange("p (j c) -> p j c", c=c1)
                                    [:, :, 0:c1:hid],
                                in1=adt.to_broadcast([P, dlt, H]),
                                op=ALU.add)
                            co += dlt
                        nc.scalar.activation(e2[:], e[:], AF.Lrelu,
                                             alpha=NEG_SLOPE)
                        nc.scalar.activation(
                            mv[:, :, 0:H],
                            e2[:].rearrange("p (j h) -> p j h", h=H),
                            AF.Exp)
                        co = 0
                        for gt, s0, dlt in segs:
                            gseg = gt[:, s0 * c1:(s0 + dlt) * c1]
                            nc.vector.tensor_tensor(
                                out=mv[:, co:co + dlt, H:wm]
                                    .rearrange("p j (h s) -> p j h s", s=hid),
                                in0=gseg.rearrange("p (j h s) -> p j h s",
                                                   h=H, s=hid),
                                in1=mv[:, co:co + dlt, 0:H]
                                    .to_broadcast([P, dlt, H, hid]),
                                op=ALU.mult)
                            co += dlt
                        pt = ps.tile([P, wm], F32, tag="acc")
                        for j in range(J):
                            nc.tensor.matmul(pt[:], lhsT=id16t[:],
                                             rhs=m[:, j * wm:(j + 1) * wm],
                                             start=(j == 0), stop=(j == J - 1))
                        # ---- finalize tile t ----
                        den = smp.tile([P, H], F32, tag="den")
                        nc.vector.tensor_scalar(out=den[:], in0=pt[:, 0:H],
                                                scalar1=1e-30, scalar2=None,
                                                op0=ALU.max)
                        rec = smp.tile([P, H], F32, tag="rec")
                        nc.vector.reciprocal(rec[:], den[:])
                        on = fin.tile([P, c1], F32, tag="on")
                        nc.vector.tensor_tensor(
                            out=on[:].rearrange("p (h c) -> p h c", c=hid),
                            in0=pt[:, H:wm].rearrange("p (h c) -> p h c", c=hid),
                            in1=rec[:].to_broadcast([P, H, hid]),
                            op=ALU.mult)
                        ptT = ps2.tile([P, P], F32, tag="pT")
                        nc.tensor.transpose(ptT[:], on[:], idt[:])
                        tT = fin.tile([c1, P], F32, tag="tT")
                        nc.vector.tensor_copy(tT[:], ptT[:])
                        p3 = ps3.tile([c1, P], F32, tag="p3")
                        nc.tensor.matmul(p3[:], lhsT=riv[:], rhs=tT[:],
                                         start=True, stop=True)
                        o1 = fin.tile([c1, P], F32, tag="o1")
                        nc.scalar.activation(o1[:], p3[:], AF.Relu,
                                             bias=b1t[:, 0:1])
                        p4 = ps4.tile([P, cfg.out_c + 2], F32, tag="p4")
                        nc.tensor.matmul(p4[:], lhsT=o1[:], rhs=w2t[:],
                                         start=True, stop=True)
                        h2 = h2big[:, ti * cfg.row2:(ti + 1) * cfg.row2]
                        nc.vector.memset(h2, 0.0)
                        nc.vector.tensor_copy(h2[:, 0:cfg.out_c + 2], p4[:])
                    t0 = pair[0]
                    nc.sync.dma_start(
                        out=h2rows[:, t0 * cfg.row2:(t0 + npair) * cfg.row2],
                        in_=h2big[:])
    nc.compile()
    return nc


# ----------------------------------------------------------------------------
# launch C: layer-2 message passing + log_softmax
# ----------------------------------------------------------------------------
def build_launchC(cfg, d0t, d1t, stot16, nhalf, rep=1):
    oc = cfg.out_c
    wm = 1 + oc
    r2 = cfg.row2
    nc = bacc.Bacc("TRN2", target_bir_lowering=False, debug=False,
                   num_devices=cfg.ncores, num_swdge_queues=4)
    tb0 = nc.dram_tensor("tb0", [nhalf, r2], F32, kind="ExternalInput")
    tb1 = nc.dram_tensor("tb1", [nhalf, r2], F32, kind="ExternalInput")
    idxs = nc.dram_tensor("idxs", [P, stot16 // 16], I16, kind="ExternalInput")
    adsw = nc.dram_tensor("adsw", [P, cfg.ntiles], F32, kind="ExternalInput")
    id16 = nc.dram_tensor("id16", [P, P], F16, kind="ExternalInput")
    b2c = nc.dram_tensor("b2c", [P, oc], F32, kind="ExternalInput")
    outr = nc.dram_tensor("outr", [P, cfg.ntiles * oc], F32,
                          kind="ExternalOutput")

    dts = (d0t, d1t)
    pairs = tile_pairs(cfg.ntiles)
    with tile.TileContext(nc) as tc:
        with tc.tile_pool(name="fix", bufs=1) as fix, \
             tc.tile_pool(name="gp", bufs=6) as gp, \
             tc.tile_pool(name="mp", bufs=4) as mp, \
             tc.tile_pool(name="ep", bufs=6) as ep, \
             tc.tile_pool(name="sm", bufs=8) as smp, \
             tc.tile_pool(name="fin", bufs=3) as fin, \
             tc.tile_pool(name="ps", bufs=2, space="PSUM") as ps:
            it = fix.tile([P, stot16 // 16], I16)
            nc.sync.dma_start(out=it[:], in_=idxs[:, :])
            ad = fix.tile([P, cfg.ntiles], F32)
            nc.sync.dma_start(out=ad[:], in_=adsw[:, :])
            id16t = fix.tile([P, P], F16)
            nc.sync.dma_start(out=id16t[:], in_=id16[:, :])
            b2t = fix.tile([P, oc], F32)
            nc.sync.dma_start(out=b2t[:], in_=b2c[:, :])

            qi = [0]
            for _ in range(rep):
                pos = 0
                for pair in pairs:
                    gts = {}
                    segstart = {}
                    for h in (0, 1):
                        tbl = tb0 if h == 0 else tb1
                        ds = [int(dts[h][t]) for t in pair]
                        dtot = sum(ds)
                        segstart[h] = [sum(ds[:i]) for i in range(len(pair))]
                        if dtot == 0:
                            gts[h] = None
                            continue
                        gt = gp.tile([P, dtot * r2], F32, tag=f"g{h}")
                        nsp = 3 if dtot >= 3 else dtot
                        bnds = [dtot * i // nsp for i in range(nsp + 1)]
                        for si in range(nsp):
                            a, b = bnds[si], bnds[si + 1]
                            if b == a:
                                continue
                            nc.gpsimd.dma_gather(
                                out_ap=gt[:, a * r2:b * r2]
                                    .rearrange("p (c e) -> p c e", e=r2),
                                in_ap=tbl[:, :],
                                idxs_ap=it[:, (pos + a * P) // 16:
                                           (pos + b * P) // 16],
                                num_idxs=(b - a) * P,
                                num_idxs_reg=(b - a) * P,
                                elem_size=r2,
                                single_packet=False,
                                queue_num=qi[0] % 4,
                            )
                            qi[0] += 1
                        pos += dtot * P
                        gts[h] = gt
                    # ---- pair-merged M: cols [tile0: ex|agg, tile1: ex|agg]
                    npair = len(pair)
                    wmP = npair * wm
                    Js = [int(d0t[t] + d1t[t]) for t in pair]
                    Jmax = max(Js)
                    m = mp.tile([P, Jmax * wmP], F16, tag="m")
                    mv = m[:].rearrange("p (j w) -> p j w", w=wmP)
                    for ti, t in enumerate(pair):
                        J = Js[ti]
                        segs = []
                        for h in (0, 1):
                            dlt = int(dts[h][t])
                            if dlt:
                                segs.append((gts[h], segstart[h][ti], dlt))
                        e = ep.tile([P, J], F32, tag="e")
                        e2 = ep.tile([P, J], F32, tag="e2")
                        co = 0
                        for gt, s0, dlt in segs:
                            gseg = gt[:, s0 * r2:(s0 + dlt) * r2]
                            nc.vector.tensor_scalar(
                                out=e[:, co:co + dlt],
                                in0=gseg.rearrange("p (j r) -> p j r", r=r2)
                                    [:, :, oc:oc + 1]
                                    .rearrange("p j o -> p (j o)"),
                                scalar1=ad[:, t:t + 1], scalar2=None,
                                op0=ALU.add)
                            co += dlt
                        nc.scalar.activation(e2[:], e[:], AF.Lrelu,
                                             alpha=NEG_SLOPE)
                        nc.scalar.activation(
                            mv[:, 0:J, ti * wm:ti * wm + 1]
                                .rearrange("p j o -> p (j o)"),
                            e2[:], AF.Exp)
                        co = 0
                        for gt, s0, dlt in segs:
                            gseg = gt[:, s0 * r2:(s0 + dlt) * r2]
                            nc.vector.tensor_tensor(
                                out=mv[:, co:co + dlt,
                                       ti * wm + 1:ti * wm + wm],
                                in0=gseg.rearrange("p (j r) -> p j r", r=r2)
                                    [:, :, 0:oc],
                                in1=mv[:, co:co + dlt, ti * wm:ti * wm + 1]
                                    .rearrange("p j o -> p (j o)")
                                    .to_broadcast([P, dlt, oc]),
                                op=ALU.mult)
                            co += dlt
                        if J < Jmax:
                            nc.vector.memset(
                                mv[:, J:Jmax, ti * wm:(ti + 1) * wm], 0.0)
                    pt = ps.tile([P, wmP], F32, tag="acc")
                    for j in range(Jmax):
                        nc.tensor.matmul(pt[:], lhsT=id16t[:],
                                         rhs=m[:, j * wmP:(j + 1) * wmP],
                                         start=(j == 0), stop=(j == Jmax - 1))
                    # ---- batched finalize: divide, +b2, log_softmax ----
                    ptv = pt[:].rearrange("p (t w) -> p t w", w=wm)
                    den = smp.tile([P, npair], F32, tag="den")
                    nc.vector.tensor_scalar(
                        out=den[:],
                        in0=ptv[:, :, 0:1].rearrange("p t o -> p (t o)"),
                        scalar1=1e-30, scalar2=None, op0=ALU.max)
                    rec = smp.tile([P, npair], F32, tag="rec")
                    nc.vector.reciprocal(rec[:], den[:])
                    o2b = fin.tile([P, npair * oc], F32, tag="o2b")
                    o2bv = o2b[:].rearrange("p (t c) -> p t c", c=oc)
                    nc.vector.tensor_tensor(
                        out=o2bv, in0=ptv[:, :, 1:wm],
                        in1=rec[:].to_broadcast([P, npair, oc]),
                        op=ALU.mult)
                    nc.vector.tensor_tensor(
                        out=o2bv, in0=o2bv,
                        in1=b2t[:].rearrange("p (o c) -> p o c", o=1)
                            .to_broadcast([P, npair, oc]),
                        op=ALU.add)
                    mx = smp.tile([P, npair], F32, tag="mx")
                    nc.vector.tensor_reduce(out=mx[:], in_=o2bv,
                                            axis=mybir.AxisListType.X,
                                            op=ALU.max)
                    xs = fin.tile([P, npair * oc], F32, tag="xs")
                    xsv = xs[:].rearrange("p (t c) -> p t c", c=oc)
                    nc.vector.tensor_tensor(
                        out=xsv, in0=o2bv,
                        in1=mx[:].to_broadcast([P, npair, oc]),
                        op=ALU.subtract)
                    ex = fin.tile([P, npair * oc], F32, tag="ex")
                    nc.scalar.activation(ex[:], xs[:], AF.Exp)
                    se = smp.tile([P, npair], F32, tag="se")
                    nc.vector.tensor_reduce(
                        out=se[:], in_=ex[:].rearrange("p (t c) -> p t c", c=oc),
                        axis=mybir.AxisListType.X, op=ALU.add)
                    ls = smp.tile([P, npair], F32, tag="ls")
                    nc.scalar.activation(ls[:], se[:], AF.Ln)
                    fo = fin.tile([P, npair * oc], F32, tag="fo")
                    nc.vector.tensor_tensor(
                        out=fo[:].rearrange("p (t c) -> p t c", c=oc),
                        in0=xsv, in1=ls[:].to_broadcast([P, npair, oc]),
                        op=ALU.subtract)
                    t0 = pair[0]
                    nc.sync.dma_start(
                        out=outr[:, t0 * oc:(t0 + npair) * oc], in_=fo[:])
    nc.compile()
    return nc


# ----------------------------------------------------------------------------
# full pipeline
# ----------------------------------------------------------------------------
def run_gat(cfg, inputs, timing=False):
    x = np.asarray(inputs["x"], dtype=np.float32)
    edge_index = np.asarray(inputs["edge_index"])
    W1e, Rinv, W2e = make_consts(
        cfg, np.asarray(inputs["W1"], np.float64),
        np.asarray(inputs["a1_src"], np.float64),
        np.asarray(inputs["a1_dst"], np.float64),
        np.asarray(inputs["W2"], np.float64),
        np.asarray(inputs["a2_src"], np.float64),
        np.asarray(inputs["a2_dst"], np.float64))
    b1 = np.asarray(inputs["b1"], np.float32)
    b2 = np.asarray(inputs["b2"], np.float32)
    pre = preprocess(cfg, edge_index)
    C = cfg.ncores

    # ---- launch A ----
    ncA = build_launchA(cfg)
    rA = SpmdRunner(ncA, C)
    mapsA = []
    for c in range(C):
        g = pre["gids"][c]
        xp = np.zeros((cfg.npad, cfg.in_c), np.float32)
        valid = g >= 0
        xp[np.flatnonzero(valid)] = x[g[valid]]
        mapsA.append({"xT": np.ascontiguousarray(xp.T), "W1e": W1e})
    outsA = rA.results(rA.run(rA.put_inputs(mapsA)))

    # assemble h~ table (fp16) + alpha_d (pi-order per core)
    tblg = np.zeros((cfg.N, cfg.c1), np.float16)
    adsws = []
    for c in range(C):
        g = pre["gids"][c]
        valid = g >= 0
        hr = outsA[c]["hrows"].reshape(P, cfg.ntiles, cfg.c1) \
            .transpose(1, 0, 2).reshape(cfg.npad, cfg.c1)
        tblg[g[valid]] = hr[np.flatnonzero(valid)]
        adsws.append(outsA[c]["adrows"])  # already [P, ntiles*H] packed
    s0, s1 = pre["sloc0"], pre["sloc1"]
    nh = pre["tabrows"]
    tb0 = np.zeros((nh, cfg.c1), np.float16)
    tb1 = np.zeros((nh, cfg.c1), np.float16)
    w0 = s0 >= 0
    tb0[s0[w0]] = tblg[w0]
    w1 = s1 >= 0
    tb1[s1[w1]] = tblg[w1]
    for tb in (tb0, tb1):
        tb[pre["dummy"], 0:cfg.c1:cfg.hid] = DUMMY_ALPHA
    ident = np.eye(P, dtype=np.float32)
    id16 = np.eye(P, dtype=np.float16)

    # ---- launch B ----
    ncB = build_launchB(cfg, pre["d0t"], pre["d1t"], pre["stot16"], nh)
    rB = SpmdRunner(ncB, C)
    mapsB = [{"tb0": tb0, "tb1": tb1, "idxs": pre["idxws"][c],
              "adsw": adsws[c], "ident": ident, "id16": id16, "rinv": Rinv,
              "w2e": W2e, "b1c": b1.reshape(-1, 1)} for c in range(C)]
    outsB = rB.results(rB.run(rB.put_inputs(mapsB)))

    # assemble h2~ table + alpha2_d
    tbl2g = np.zeros((cfg.N, cfg.row2), np.float32)
    ad2sws = []
    for c in range(C):
        g = pre["gids"][c]
        valid = g >= 0
        h2r = outsB[c]["h2rows"].reshape(P, cfg.ntiles, cfg.row2) \
            .transpose(1, 0, 2).reshape(cfg.npad, cfg.row2)
        row = np.zeros((cfg.npad, cfg.row2), np.float32)
        row[:, 0:cfg.out_c + 1] = h2r[:, 0:cfg.out_c + 1]
        tbl2g[g[valid]] = row[np.flatnonzero(valid)]
        ad2 = h2r[:, cfg.out_c + 1]  # [npad] pi-order
        ad2sws.append(np.ascontiguousarray(
            ad2.reshape(cfg.ntiles, P).T))
    tb20 = np.zeros((nh, cfg.row2), np.float32)
    tb21 = np.zeros((nh, cfg.row2), np.float32)
    tb20[s0[w0]] = tbl2g[w0]
    tb21[s1[w1]] = tbl2g[w1]
    for tb in (tb20, tb21):
        tb[pre["dummy"], cfg.out_c] = DUMMY_ALPHA

    # ---- launch C ----
    ncC = build_launchC(cfg, pre["d0t"], pre["d1t"], pre["stot16"], nh)
    rC = SpmdRunner(ncC, C)
    b2bc = np.tile(b2.reshape(1, -1), (P, 1)).astype(np.float32)
    mapsC = [{"tb0": tb20, "tb1": tb21, "idxs": pre["idxws"][c],
              "adsw": ad2sws[c], "id16": id16, "b2c": b2bc}
             for c in range(C)]
    outsC = rC.results(rC.run(rC.put_inputs(mapsC)))

    out = np.zeros((cfg.N, cfg.out_c), np.float32)
    for c in range(C):
        g = pre["gids"][c]
        valid = g >= 0
        our = outsC[c]["outr"].reshape(P, cfg.ntiles, cfg.out_c) \
            .transpose(1, 0, 2).reshape(cfg.npad, cfg.out_c)
        out[g[valid]] = our[np.flatnonzero(valid)]
    return out


def kernel(**inputs) -> np.ndarray:
    return run_gat(CFG, inputs)


# revision 18
# speedup vs baseline: 2.0961x; 2.0961x over previous
"""2-layer GAT on 8 Trainium2 NeuronCores (Bass/Tile, SPMD via axon PJRT).

Strategy (dst-sharded message passing, 3 launches, no collectives):
  A: per-core feature transform of its node shard: h~ = x @ (W1 R) and
     alpha_dst = x @ (W1 A1d). R is a per-head invertible rotation whose
     first column is a1_src, so alpha_src of a gathered row is just its
     strided column 16h -- no separate alpha table gather needed. h~ is
     stored fp16 (halves gather traffic in B).
  B: layer-1 message passing. Edge slots laid out node-major per 128-node
     destination tile (slot j of node p = chunk j, partition p), so the
     PSUM-accumulating matmul uses a constant identity lhsT (fp16: 1 cyc/
     row vs fp32's 4). Slot streams are grouped per PAIR of tiles and per
     half-table so each (pair, half) is ONE dma_gather. All per-edge
     elementwise math is batched per tile (one DVE/ACT instruction over
     all J chunks) instead of per chunk: e = g[:,0:128:16] + a_d; Lrelu;
     Exp -> M[:,:,0:8] (denominator cols, fp16); M[:,:,8:136] = g * ex
     (fp16). J matmuls accumulate denom+agg into PSUM. Finalize: divide,
     un-rotate (PE transpose + matmul by R^-1), +b1, ReLU, then fused
     layer-2 transform h2~ = relu(h1) @ [W2 | W2 a2s | W2 a2d] -> DRAM.
  C: layer-2 message passing over the same slot structure (row =
     [h2(40) | alpha2_src | pad] fp32, 256B), same batched-elementwise
     scheme, finalize with divide, +b2, log_softmax.
Host does only sharding glue: edge partitioning/sorting, half balancing,
permutations, table assembly between launches, constants.
"""
import sys
sys.path.insert(0, "/opt/trn_rl_repo")

import numpy as np
import jax

import concourse.bass as bass
import concourse.tile as tile
import concourse.mybir as mybir
from concourse import bacc
from concourse.bass2jax import _bass_exec_p, partition_id_tensor, install_neuronx_cc_hook
from jax.sharding import Mesh, PartitionSpec
from jax.experimental.shard_map import shard_map

F32 = mybir.dt.float32
F16 = mybir.dt.float16
I16 = mybir.dt.int16
AF = mybir.ActivationFunctionType
ALU = mybir.AluOpType

NEG_SLOPE = 0.2
DUMMY_ALPHA = -30000.0
P = 128


# ----------------------------------------------------------------------------
# configuration (sizes hardcoded for the graded problem; small configs used by
# the self-test harness pass explicit cfg)
# ----------------------------------------------------------------------------
class Cfg:
    def __init__(self, N, E, in_c=128, hid=16, heads=8, out_c=40, ncores=8):
        self.N, self.E = N, E
        self.in_c, self.hid, self.heads, self.out_c = in_c, hid, heads, out_c
        self.ncores = ncores
        self.npc = N // ncores                      # real nodes per core
        self.ntiles = -(-self.npc // P)             # dst tiles per core
        self.npad = self.ntiles * P                 # padded nodes per core
        # source table halves: node table slot range, dummy at local HALF
        tot = N
        self.half = -(-tot // 2)
        self.half = ((self.half + P - 1) // P) * P  # round half size up
        assert self.half + 1 <= 32767, "int16 gather index limit"
        self.c1 = heads * hid                       # layer-1 out channels (128)
        self.row2 = 64                              # layer-2 table row elems


CFG = Cfg(N=50000, E=800000)


def tile_pairs(ntiles):
    return [tuple(t for t in (g, g + 1) if t < ntiles)
            for g in range(0, ntiles, 2)]


# ----------------------------------------------------------------------------
# host-side math constants
# ----------------------------------------------------------------------------
def householder_rot(a):
    """R [k,k] with R[:,0] = a exactly, other columns orthonormal; plus R^-1."""
    k = a.shape[0]
    a = a.astype(np.float64)
    s = np.linalg.norm(a)
    if s < 1e-30:
        R = np.eye(k)
        R[0, 0] = 1.0
        return R, np.linalg.inv(R)
    u = a / s
    if u[0] > 1.0 - 1e-12:
        H = np.eye(k)
    else:
        v = u - np.eye(k)[:, 0]
        H = np.eye(k) - 2.0 * np.outer(v, v) / (v @ v)
    R = H @ np.diag([s] + [1.0] * (k - 1))
    Rinv = np.diag([1.0 / s] + [1.0] * (k - 1)) @ H
    return R, Rinv


def make_consts(cfg, W1, a1_src, a1_dst, W2, a2_src, a2_dst):
    """W1ext [in_c, c1+heads], Rinv_bd [c1, c1], W2ext [c1, out_c+2]."""
    H, D = cfg.heads, cfg.hid
    Rbd = np.zeros((cfg.c1, cfg.c1))
    Rinv = np.zeros((cfg.c1, cfg.c1))
    for h in range(H):
        R_h, Rinv_h = householder_rot(a1_src[h].astype(np.float64))
        Rbd[h * D:(h + 1) * D, h * D:(h + 1) * D] = R_h
        Rinv[h * D:(h + 1) * D, h * D:(h + 1) * D] = Rinv_h
    A1d = np.zeros((cfg.c1, H))
    for h in range(H):
        A1d[h * D:(h + 1) * D, h] = a1_dst[h].astype(np.float64)
    W1e = np.concatenate([W1.astype(np.float64) @ Rbd,
                          W1.astype(np.float64) @ A1d], axis=1)
    W2e = np.concatenate([W2.astype(np.float64),
                          W2.astype(np.float64) @ a2_src[0].astype(np.float64)[:, None],
                          W2.astype(np.float64) @ a2_dst[0].astype(np.float64)[:, None]],
                         axis=1)
    return (W1e.astype(np.float32), Rinv.astype(np.float32),
            W2e.astype(np.float32))


# ----------------------------------------------------------------------------
# host-side graph preprocessing
# ----------------------------------------------------------------------------
def balance_halves(cfg, src, dst, cap=None):
    """Assign each node a half bit, balancing each dst's in-edges between
    halves. Chunked greedy over sources (vectorized)."""
    N = cfg.N
    order = np.argsort(src, kind="stable")
    ssrc = src[order]
    sdst = dst[order]
    starts = np.searchsorted(ssrc, np.arange(N + 1))
    imb = np.zeros(N, dtype=np.int64)
    halfbit = np.zeros(N, dtype=np.int8)
    if cap is None:
        cap = cfg.half
    cnt = [0, 0]
    K = 512
    for lo in range(0, N, K):
        hi = min(lo + K, N)
        a, b = starts[lo], starts[hi]
        if b > a:
            ridx = np.minimum(starts[lo:hi] - a, b - a - 1)
            seg = np.add.reduceat(imb[sdst[a:b]], ridx)
            empty = (starts[lo + 1:hi + 1] - starts[lo:hi]) == 0
            seg = np.where(empty, 0, seg)
        else:
            seg = np.zeros(hi - lo, dtype=np.int64)
        h = (seg > 0).astype(np.int8)
        halfbit[lo:hi] = h
        cnt[0] += int((h == 0).sum())
        cnt[1] += int((h == 1).sum())
        hb_e = halfbit[ssrc[a:b]]
        np.add.at(imb, sdst[a:b], 1 - 2 * hb_e.astype(np.int64))
    # refinement sweeps (chunked, approximate)
    for _ in range(2):
        for lo in range(0, N, K):
            hi = min(lo + K, N)
            a, b = starts[lo], starts[hi]
            if b <= a:
                continue
            ridx = np.minimum(starts[lo:hi] - a, b - a - 1)
            seg = np.add.reduceat(imb[sdst[a:b]], ridx)
            empty = (starts[lo + 1:hi + 1] - starts[lo:hi]) == 0
            seg = np.where(empty, 0, seg)
            degs = (starts[lo + 1:hi + 1] - starts[lo:hi]).astype(np.int64)
            hc = halfbit[lo:hi].astype(np.int64)
            bias_excl = seg - degs * (1 - 2 * hc)
            hn = (bias_excl > 0).astype(np.int8)
            changed = hn != halfbit[lo:hi]
            if not changed.any():
                continue
            cnt[0] += int((hn[changed] == 0).sum()) - int((halfbit[lo:hi][changed] == 0).sum())
            cnt[1] += int((hn[changed] == 1).sum()) - int((halfbit[lo:hi][changed] == 1).sum())
            halfbit[lo:hi] = hn
            hb_e_new = halfbit[ssrc[a:b]].astype(np.int64)
            # recompute imb contribution for edges in this block
            d = sdst[a:b]
            hb_e_old = np.repeat(hc, degs)
            np.add.at(imb, d, (1 - 2 * hb_e_new) - (1 - 2 * hb_e_old))
    # enforce capacity
    for hh in (0, 1):
        over = cnt[hh] - cap
        if over > 0:
            ids = np.flatnonzero(halfbit == hh)[:over]
            halfbit[ids] = 1 - hh
            cnt[hh] -= over
            cnt[1 - hh] += over
    return halfbit


def preprocess(cfg, edge_index):
    """Build all per-core edge-slot structures. Returns dict.

    High-out-degree sources are REPLICATED into both half tables (spare
    int16-index capacity), making their edges' half assignment free; the
    free edges are then used to split every destination's in-edges near
    ceil/floor between the two halves, minimizing per-tile max-degree
    padding."""
    N, E, C = cfg.N, cfg.E, cfg.ncores
    loop = np.arange(N, dtype=np.int64)
    src = np.concatenate([edge_index[0], loop]).astype(np.int64)
    dst = np.concatenate([edge_index[1], loop]).astype(np.int64)

    # --- choose replicated sources (top out-degree) ---
    TABMAX = 32767  # int16 gather-index limit (incl. dummy row)
    R = min(N // 3, max(0, 2 * (TABMAX - 1) - N - 2))
    outdeg = np.bincount(src, minlength=N)
    rep = np.zeros(N, bool)
    if R > 0:
        rep[np.argsort(-outdeg, kind="stable")[:R]] = True
    nonrep = ~rep
    # halfbit for non-replicated sources (balance the fixed part)
    em = nonrep[src]
    nr_cap = TABMAX - 1 - R
    halfbit = balance_halves(cfg, src[em], dst[em], cap=nr_cap)
    halfbit[rep] = 0  # unused for replicated
    # enforce per-half capacity over non-replicated nodes only
    for hh in (0, 1):
        ids = np.flatnonzero(nonrep & (halfbit == hh))
        over = len(ids) - nr_cap
        if over > 0:
            halfbit[ids[:over]] = 1 - hh
    # violation-targeted refinement: flip fixed sources whose edges push a
    # destination's fixed count past its ceil/floor share (the free edges of
    # replicated sources can only absorb imbalance up to their count)
    degd_all = np.bincount(dst, minlength=N)
    ceil_d = (degd_all + 1) // 2
    floor_d = degd_all - ceil_d
    sfix = src[em]
    dfix = dst[em]
    order_s = np.argsort(sfix, kind="stable")
    sfs, dfs = sfix[order_s], dfix[order_s]
    sstarts = np.searchsorted(sfs, np.arange(N + 1))
    nev = len(sfs)
    rng = np.random.default_rng(0)
    for _ in range(12):
        hbe = halfbit[sfix]
        n0 = np.zeros(N, np.int64)
        n1 = np.zeros(N, np.int64)
        np.add.at(n0, dfix, 1 - hbe)
        np.add.at(n1, dfix, hbe)
        peg01 = ((n0 > ceil_d)[dfs].astype(np.int64)
                 - (n1 >= floor_d)[dfs].astype(np.int64))
        peg10 = ((n1 > floor_d)[dfs].astype(np.int64)
                 - (n0 >= ceil_d)[dfs].astype(np.int64))
        ridx = np.minimum(sstarts[:-1], max(nev - 1, 0))
        g01 = np.add.reduceat(peg01, ridx)
        g10 = np.add.reduceat(peg10, ridx)
        emptys = np.diff(sstarts) == 0
        g01 = np.where(emptys, 0, g01)
        g10 = np.where(emptys, 0, g10)
        sub = rng.random(N) < 0.4
        halfbit[(halfbit == 0) & nonrep & (g01 > 0) & sub] = 1
        halfbit[(halfbit == 1) & nonrep & (g10 > 0) & sub] = 0
        for hh in (0, 1):
            ids = np.flatnonzero(nonrep & (halfbit == hh))
            over = len(ids) - nr_cap
            if over > 0:
                halfbit[ids[:over]] = 1 - hh

    # --- table slots: nonrep nodes first (id order), then replicated ---
    sloc0 = np.full(N, -1, np.int64)
    sloc1 = np.full(N, -1, np.int64)
    m0 = nonrep & (halfbit == 0)
    m1 = nonrep & (halfbit == 1)
    sloc0[m0] = np.arange(int(m0.sum()))
    sloc1[m1] = np.arange(int(m1.sum()))
    repn = np.flatnonzero(rep)
    sloc0[repn] = int(m0.sum()) + np.arange(len(repn))
    sloc1[repn] = int(m1.sum()) + np.arange(len(repn))
    rows0 = int(m0.sum()) + len(repn)
    rows1 = int(m1.sum()) + len(repn)
    tabrows = max(rows0, rows1) + 1
    assert tabrows <= TABMAX + 1
    dummy = tabrows - 1

    # snake-deal sharding: global degree-desc order, node i -> core i%C,
    # rank i//C. gids[c][r] = global node id at (core c, rank r).
    degg = np.bincount(dst, minlength=N)
    gorder = np.argsort(-degg, kind="stable")
    node2core = np.empty(N, np.int32)
    node2rank = np.empty(N, np.int32)
    node2core[gorder] = (np.arange(N) % C).astype(np.int32)
    node2rank[gorder] = (np.arange(N) // C).astype(np.int32)
    gids = []
    for c in range(C):
        g = np.full(cfg.npad, -1, np.int64)
        sel = gorder[c::C]
        g[:len(sel)] = sel
        gids.append(g)

    # --- per-edge half: fixed for nonrep sources, balanced fill for rep ---
    order_d = np.argsort(dst, kind="stable")
    so, do = src[order_d], dst[order_d]
    isfree = rep[so]
    fhalf = np.where(isfree, 0, halfbit[so]).astype(np.int64)
    starts_d = np.searchsorted(do, np.arange(N + 1))
    degd = np.diff(starts_d)
    ne = len(so)
    fixed0ct = np.zeros(N, np.int64)
    fixed1ct = np.zeros(N, np.int64)
    np.add.at(fixed0ct, do[~isfree], 1 - fhalf[~isfree])
    np.add.at(fixed1ct, do[~isfree], fhalf[~isfree])
    f = degd - fixed0ct - fixed1ct
    h0t = np.clip((degd + 1) // 2, fixed0ct, degd - fixed1ct)
    free0 = h0t - fixed0ct
    gs = np.repeat(starts_d[:-1], degd)
    base = np.cumsum(isfree) - isfree
    jfree = base - base[gs]
    ehalf_o = np.where(isfree, (jfree >= free0[do]).astype(np.int64), fhalf)
    ehalf = np.empty(ne, np.int64)
    ehalf[order_d] = ehalf_o
    esloc = np.where(ehalf == 0, sloc0[src], sloc1[src])

    deg_hc = np.zeros((C, cfg.npad, 2), dtype=np.int32)
    np.add.at(deg_hc, (node2core[dst], node2rank[dst], ehalf), 1)

    # common per-tile deltas across cores (same program on all cores)
    d0t = deg_hc[:, :, 0].reshape(C, cfg.ntiles, P).max(axis=(0, 2)).astype(np.int32)
    d1t = deg_hc[:, :, 1].reshape(C, cfg.ntiles, P).max(axis=(0, 2)).astype(np.int32)
    stot = int(128 * (d0t.sum() + d1t.sum()))
    stot16 = ((stot + 15) // 16) * 16

    # slot base position of each (tile, half): pair-grouped stream order
    # [pair][half][tile-in-pair] so each (pair, half) is one gather.
    dts = (d0t, d1t)
    bases = np.zeros((cfg.ntiles, 2), dtype=np.int64)
    block_rank = np.zeros((cfg.ntiles, 2), dtype=np.int64)
    pos = 0
    br = 0
    for pair in tile_pairs(cfg.ntiles):
        for h in (0, 1):
            for t in pair:
                bases[t, h] = pos
                block_rank[t, h] = br
                pos += 128 * int(dts[h][t])
                br += 1
    assert pos == stot

    idx_flats = []
    for c in range(C):
        m = node2core[dst] == c
        hb = ehalf[m]
        vs = esloc[m]
        r = node2rank[dst[m]].astype(np.int64)
        t = r // P
        part = r % P
        key = block_rank[t, hb] * cfg.npad + r
        order = np.argsort(key, kind="stable")
        ks = key[order]
        # occurrence j of each edge within its (node, half) group
        starts_ = np.flatnonzero(np.concatenate([[1], ks[1:] != ks[:-1]]))
        gstart = np.repeat(starts_,
                           np.diff(np.concatenate([starts_, [len(ks)]])))
        j = np.arange(len(ks)) - gstart
        pos_ = (bases[t[order], hb[order]] + j * 128 + part[order])
        idx_flat = np.full(stot16, dummy, dtype=np.int16)
        idx_flat[pos_] = vs[order].astype(np.int16)
        idx_flats.append(idx_flat)

    # wrap to [128, stot16//16] with 8x replication
    idxws = []
    for c in range(C):
        w = np.zeros((P, stot16 // 16), dtype=np.int16)
        i = np.arange(stot16)
        w[i % 16, i // 16] = idx_flats[c]
        for r_ in range(1, 8):
            w[r_ * 16:(r_ + 1) * 16] = w[:16]
        idxws.append(w)

    return dict(src=src, dst=dst, sloc0=sloc0, sloc1=sloc1, gids=gids,
                tabrows=tabrows, dummy=dummy,
                d0t=d0t, d1t=d1t, bases=bases, stot16=stot16, idxws=idxws)


# ----------------------------------------------------------------------------
# SPMD runner (cached jit, modeled on bass2jax.run_bass_via_pjrt)
# ----------------------------------------------------------------------------
class SpmdRunner:
    def __init__(self, nc, n_cores):
        install_neuronx_cc_hook()
        self.nc, self.n_cores = nc, n_cores
        pname = nc.partition_id_tensor.name if nc.partition_id_tensor else None
        in_names, out_names, out_avals, zero_outs = [], [], [], []
        for alloc in nc.m.functions[0].allocations:
            if not isinstance(alloc, mybir.MemoryLocationSet):
                continue
            name = alloc.memorylocations[0].name
            if alloc.kind == "ExternalInput":
                if name != pname:
                    in_names.append(name)
            elif alloc.kind == "ExternalOutput":
                out_names.append(name)
                shape = tuple(alloc.tensor_shape)
                dtype = mybir.dt.np(alloc.dtype)
                out_avals.append(jax.core.ShapedArray(shape, dtype))
                zero_outs.append(np.zeros(shape, dtype))
        self.n_params, self.in_names, self.out_names = len(in_names), in_names, out_names
        self.zero_outs = zero_outs
        all_in = in_names + out_names + ([pname] if pname else [])

        def _body(*args):
            ops = list(args)
            if pname is not None:
                ops.append(partition_id_tensor())
            return tuple(_bass_exec_p.bind(
                *ops, out_avals=tuple(out_avals), in_names=tuple(all_in),
                out_names=tuple(out_names), lowering_input_output_aliases=(),
                sim_require_finite=False, sim_require_nnan=False, nc=nc))

        donate = tuple(range(self.n_params, self.n_params + len(out_names)))
        devices = jax.devices()[:n_cores]
        mesh = Mesh(np.asarray(devices), ("core",))
        ispec = (PartitionSpec("core"),) * (self.n_params + len(out_names))
        ospec = (PartitionSpec("core"),) * len(out_names)
        self.fn = jax.jit(shard_map(_body, mesh=mesh, in_specs=ispec,
                                    out_specs=ospec, check_rep=False),
                          donate_argnums=donate, keep_unused=True)

    def put_inputs(self, in_maps):
        concat = [np.concatenate([np.asarray(m[n]) for m in in_maps], axis=0)
                  for n in self.in_names]
        return [jax.device_put(x) for x in concat]

    def run(self, dev_inputs, retries=2):
        import time as _time
        for att in range(retries + 1):
            try:
                zeros = [np.concatenate([z] * self.n_cores, axis=0)
                         for z in self.zero_outs]
                outs = self.fn(*dev_inputs, *zeros)
                jax.block_until_ready(outs)
                return outs
            except Exception:
                if att == retries:
                    raise
                _time.sleep(60)

    def results(self, outs):
        res = [dict() for _ in range(self.n_cores)]
        for i, name in enumerate(self.out_names):
            for c, part in enumerate(np.split(np.asarray(outs[i]), self.n_cores)):
                res[c][name] = part
        return res


# ----------------------------------------------------------------------------
# launch A: h~ = x @ W1ext (per-core shard, pi-order); hrows in fp16
# ----------------------------------------------------------------------------
def build_launchA(cfg, rep=1):
    nc = bacc.Bacc("TRN2", target_bir_lowering=False, debug=False,
                   num_devices=cfg.ncores)
    w = cfg.c1 + cfg.heads
    H = cfg.heads
    xT = nc.dram_tensor("xT", [cfg.in_c, cfg.npad], F32, kind="ExternalInput")
    W1e = nc.dram_tensor("W1e", [cfg.in_c, w], F32, kind="ExternalInput")
    hrows = nc.dram_tensor("hrows", [P, cfg.ntiles * cfg.c1], F16,
                           kind="ExternalOutput")
    adrows = nc.dram_tensor("adrows", [P, cfg.ntiles * cfg.heads], F32,
                            kind="ExternalOutput")
    GRP = 8
    with tile.TileContext(nc) as tc:
        with tc.tile_pool(name="fix", bufs=1) as fix, \
             tc.tile_pool(name="xb", bufs=2) as xb, \
             tc.tile_pool(name="ob", bufs=2) as ob, \
             tc.tile_pool(name="ps", bufs=4, space="PSUM") as ps:
            wt = fix.tile([cfg.in_c, w], F32)
            nc.sync.dma_start(out=wt[:], in_=W1e[:, :])
            for _ in range(rep):
              for g in range(0, cfg.ntiles, GRP):
                  ng = min(GRP, cfg.ntiles - g)
                  xbig = xb.tile([cfg.in_c, ng * P], F32, tag="x")
                  nc.sync.dma_start(out=xbig[:],
                                    in_=xT[:, g * P:(g + ng) * P])
                  hbig = ob.tile([P, ng * cfg.c1], F16, tag="h")
                  abig = ob.tile([P, ng * H], F32, tag="a")
                  for i in range(ng):
                      pt = ps.tile([P, w], F32, tag="ps")
                      nc.tensor.matmul(pt[:], lhsT=xbig[:, i * P:(i + 1) * P],
                                       rhs=wt[:], start=True, stop=True)
                      nc.vector.tensor_copy(hbig[:, i * cfg.c1:(i + 1) * cfg.c1],
                                            pt[:, 0:cfg.c1])
                      nc.vector.tensor_copy(abig[:, i * H:(i + 1) * H],
                                            pt[:, cfg.c1:])
                  nc.sync.dma_start(
                      out=hrows[:, g * cfg.c1:(g + ng) * cfg.c1], in_=hbig[:])
                  nc.sync.dma_start(
                      out=adrows[:, g * H:(g + ng) * H], in_=abig[:])
    nc.compile()
    return nc


# ----------------------------------------------------------------------------
# launch B: layer-1 message passing + fused layer-2 feature transform
# ----------------------------------------------------------------------------
def build_launchB(cfg, d0t, d1t, stot16, nhalf, rep=1):
    H = cfg.heads
    c1 = cfg.c1
    hid = cfg.hid
    wm = H + c1          # M columns: [ex(H) | g*ex(c1)]
    nc = bacc.Bacc("TRN2", target_bir_lowering=False, debug=False,
                   num_devices=cfg.ncores, num_swdge_queues=4)
    tb0 = nc.dram_tensor("tb0", [nhalf, c1], F16, kind="ExternalInput")
    tb1 = nc.dram_tensor("tb1", [nhalf, c1], F16, kind="ExternalInput")
    idxs = nc.dram_tensor("idxs", [P, stot16 // 16], I16, kind="ExternalInput")
    adsw = nc.dram_tensor("adsw", [P, cfg.ntiles * H], F32, kind="ExternalInput")
    ident = nc.dram_tensor("ident", [P, P], F32, kind="ExternalInput")
    id16 = nc.dram_tensor("id16", [P, P], F16, kind="ExternalInput")
    rinv = nc.dram_tensor("rinv", [c1, c1], F32, kind="ExternalInput")
    w2e = nc.dram_tensor("w2e", [c1, cfg.out_c + 2], F32, kind="ExternalInput")
    b1c = nc.dram_tensor("b1c", [c1, 1], F32, kind="ExternalInput")
    h2rows = nc.dram_tensor("h2rows", [P, cfg.ntiles * cfg.row2], F32,
                            kind="ExternalOutput")

    dts = (d0t, d1t)
    pairs = tile_pairs(cfg.ntiles)
    with tile.TileContext(nc) as tc:
        with tc.tile_pool(name="fix", bufs=1) as fix, \
             tc.tile_pool(name="gp", bufs=6) as gp, \
             tc.tile_pool(name="mp", bufs=4) as mp, \
             tc.tile_pool(name="ep", bufs=6) as ep, \
             tc.tile_pool(name="sm", bufs=8) as smp, \
             tc.tile_pool(name="fin", bufs=3) as fin, \
             tc.tile_pool(name="ps", bufs=2, space="PSUM") as ps, \
             tc.tile_pool(name="ps2", bufs=2, space="PSUM") as ps2, \
             tc.tile_pool(name="ps3", bufs=2, space="PSUM") as ps3, \
             tc.tile_pool(name="ps4", bufs=2, space="PSUM") as ps4:
            it = fix.tile([P, stot16 // 16], I16)
            nc.sync.dma_start(out=it[:], in_=idxs[:, :])
            ad = fix.tile([P, cfg.ntiles * H], F32)
            nc.sync.dma_start(out=ad[:], in_=adsw[:, :])
            idt = fix.tile([P, P], F32)
            nc.sync.dma_start(out=idt[:], in_=ident[:, :])
            id16t = fix.tile([P, P], F16)
            nc.sync.dma_start(out=id16t[:], in_=id16[:, :])
            riv = fix.tile([c1, c1], F32)
            nc.sync.dma_start(out=riv[:], in_=rinv[:, :])
            w2t = fix.tile([c1, cfg.out_c + 2], F32)
            nc.sync.dma_start(out=w2t[:], in_=w2e[:, :])
            b1t = fix.tile([c1, 1], F32)
            nc.sync.dma_start(out=b1t[:], in_=b1c[:, :])

            qi = [0]
            for _ in range(rep):
                pos = 0
                for pair in pairs:
                    # ---- gather: one per half for the whole pair ----
                    gts = {}
                    segstart = {}
                    for h in (0, 1):
                        tbl = tb0 if h == 0 else tb1
                        ds = [int(dts[h][t]) for t in pair]
                        dtot = sum(ds)
                        segstart[h] = [sum(ds[:i]) for i in range(len(pair))]
                        if dtot == 0:
                            gts[h] = None
                            continue
                        gt = gp.tile([P, dtot * c1], F16, tag=f"g{h}")
                        nsp = 3 if dtot >= 3 else dtot
                        bnds = [dtot * i // nsp for i in range(nsp + 1)]
                        for si in range(nsp):
                            a, b = bnds[si], bnds[si + 1]
                            if b == a:
                                continue
                            nc.gpsimd.dma_gather(
                                out_ap=gt[:, a * c1:b * c1]
                                    .rearrange("p (c e) -> p c e", e=c1),
                                in_ap=tbl[:, :],
                                idxs_ap=it[:, (pos + a * P) // 16:
                                           (pos + b * P) // 16],
                                num_idxs=(b - a) * P,
                                num_idxs_reg=(b - a) * P,
                                elem_size=c1,
                                single_packet=False,
                                queue_num=qi[0] % 4,
                            )
                            qi[0] += 1
                        pos += dtot * P
                        gts[h] = gt
                    # ---- per tile batched compute ----
                    npair = len(pair)
                    h2big = fin.tile([P, npair * cfg.row2], F32, tag="h2")
                    for ti, t in enumerate(pair):
                        J = int(d0t[t] + d1t[t])
                        segs = []
                        for h in (0, 1):
                            dlt = int(dts[h][t])
                            if dlt:
                                segs.append((gts[h], segstart[h][ti], dlt))
                        m = mp.tile([P, J * wm], F16, tag="m")
                        mv = m[:].rearrange("p (j w) -> p j w", w=wm)
                        e = ep.tile([P, J * H], F32, tag="e")
                        e2 = ep.tile([P, J * H], F32, tag="e2")
                        adt = ad[:, t * H:(t + 1) * H] \
                            .rearrange("p (o h) -> p o h", o=1)
                        co = 0
                        for gt, s0, dlt in segs:
                            gseg = gt[:, s0 * c1:(s0 + dlt) * c1]
                            nc.vector.tensor_tensor(
                                out=e[:, co * H:(co + dlt) * H]
                                    .rearrange("p (j h) -> p j h", h=H),
                                in0=gseg.rearrange("p (j c) -> p j c", c=c1)
                                    [:, :, 0:c1:hid],
                                in1=adt.to_broadcast([P, dlt, H]),
                                op=ALU.add)
                            co += dlt
                        nc.scalar.activation(e2[:], e[:], AF.Lrelu,
                                             alpha=NEG_SLOPE)
                        nc.scalar.activation(
                            mv[:, :, 0:H],
                            e2[:].rearrange("p (j h) -> p j h", h=H),
                            AF.Exp)
                        co = 0
                        for gt, s0, dlt in segs:
                            gseg = gt[:, s0 * c1:(s0 + dlt) * c1]
                            nc.vector.tensor_tensor(
                                out=mv[:, co:co + dlt, H:wm]
                                    .rearrange("p j (h s) -> p j h s", s=hid),
                                in0=gseg.rearrange("p (j h s) -> p j h s",
                                                   h=H, s=hid),
                                in1=mv[:, co:co + dlt, 0:H]
                                    .to_broadcast([P, dlt, H, hid]),
                                op=ALU.mult)
                            co += dlt
                        pt = ps.tile([P, wm], F32, tag="acc")
                        for j in range(J):
                            nc.tensor.matmul(pt[:], lhsT=id16t[:],
                                             rhs=m[:, j * wm:(j + 1) * wm],
                                             start=(j == 0), stop=(j == J - 1))
                        # ---- finalize tile t ----
                        den = smp.tile([P, H], F32, tag="den")
                        nc.vector.tensor_scalar(out=den[:], in0=pt[:, 0:H],
                                                scalar1=1e-30, scalar2=None,
                                                op0=ALU.max)
                        rec = smp.tile([P, H], F32, tag="rec")
                        nc.vector.reciprocal(rec[:], den[:])
                        on = fin.tile([P, c1], F32, tag="on")
                        nc.vector.tensor_tensor(
                            out=on[:].rearrange("p (h c) -> p h c", c=hid),
                            in0=pt[:, H:wm].rearrange("p (h c) -> p h c", c=hid),
                            in1=rec[:].to_broadcast([P, H, hid]),
                            op=ALU.mult)
                        ptT = ps2.tile([P, P], F32, tag="pT")
                        nc.tensor.transpose(ptT[:], on[:], idt[:])
                        tT = fin.tile([c1, P], F32, tag="tT")
                        nc.vector.tensor_copy(tT[:], ptT[:])
                        p3 = ps3.tile([c1, P], F32, tag="p3")
                        nc.tensor.matmul(p3[:], lhsT=riv[:], rhs=tT[:],
                                         start=True, stop=True)
                        o1 = fin.tile([c1, P], F32, tag="o1")
                        nc.scalar.activation(o1[:], p3[:], AF.Relu,
                                             bias=b1t[:, 0:1])
                        p4 = ps4.tile([P, cfg.out_c + 2], F32, tag="p4")
                        nc.tensor.matmul(p4[:], lhsT=o1[:], rhs=w2t[:],
                                         start=True, stop=True)
                        h2 = h2big[:, ti * cfg.row2:(ti + 1) * cfg.row2]
                        nc.vector.memset(h2, 0.0)
                        nc.vector.tensor_copy(h2[:, 0:cfg.out_c + 2], p4[:])
                    t0 = pair[0]
                    nc.sync.dma_start(
                        out=h2rows[:, t0 * cfg.row2:(t0 + npair) * cfg.row2],
                        in_=h2big[:])
    nc.compile()
    return nc


# ----------------------------------------------------------------------------
# launch C: layer-2 message passing + log_softmax
# ----------------------------------------------------------------------------
def build_launchC(cfg, d0t, d1t, stot16, nhalf, rep=1):
    oc = cfg.out_c
    wm = 1 + oc
    r2 = cfg.row2
    nc = bacc.Bacc("TRN2", target_bir_lowering=False, debug=False,
                   num_devices=cfg.ncores, num_swdge_queues=4)
    tb0 = nc.dram_tensor("tb0", [nhalf, r2], F32, kind="ExternalInput")
    tb1 = nc.dram_tensor("tb1", [nhalf, r2], F32, kind="ExternalInput")
    idxs = nc.dram_tensor("idxs", [P, stot16 // 16], I16, kind="ExternalInput")
    adsw = nc.dram_tensor("adsw", [P, cfg.ntiles], F32, kind="ExternalInput")
    id16 = nc.dram_tensor("id16", [P, P], F16, kind="ExternalInput")
    b2c = nc.dram_tensor("b2c", [P, oc], F32, kind="ExternalInput")
    outr = nc.dram_tensor("outr", [P, cfg.ntiles * oc], F32,
                          kind="ExternalOutput")

    dts = (d0t, d1t)
    pairs = tile_pairs(cfg.ntiles)
    with tile.TileContext(nc) as tc:
        with tc.tile_pool(name="fix", bufs=1) as fix, \
             tc.tile_pool(name="gp", bufs=6) as gp, \
             tc.tile_pool(name="mp", bufs=4) as mp, \
             tc.tile_pool(name="ep", bufs=6) as ep, \
             tc.tile_pool(name="sm", bufs=8) as smp, \
             tc.tile_pool(name="fin", bufs=3) as fin, \
             tc.tile_pool(name="ps", bufs=2, space="PSUM") as ps:
            it = fix.tile([P, stot16 // 16], I16)
            nc.sync.dma_start(out=it[:], in_=idxs[:, :])
            ad = fix.tile([P, cfg.ntiles], F32)
            nc.sync.dma_start(out=ad[:], in_=adsw[:, :])
            id16t = fix.tile([P, P], F16)
            nc.sync.dma_start(out=id16t[:], in_=id16[:, :])
            b2t = fix.tile([P, oc], F32)
            nc.sync.dma_start(out=b2t[:], in_=b2c[:, :])

            qi = [0]
            for _ in range(rep):
                pos = 0
                for pair in pairs:
                    gts = {}
                    segstart = {}
                    for h in (0, 1):
                        tbl = tb0 if h == 0 else tb1
                        ds = [int(dts[h][t]) for t in pair]
                        dtot = sum(ds)
                        segstart[h] = [sum(ds[:i]) for i in range(len(pair))]
                        if dtot == 0:
                            gts[h] = None
                            continue
                        gt = gp.tile([P, dtot * r2], F32, tag=f"g{h}")
                        nsp = 3 if dtot >= 3 else dtot
                        bnds = [dtot * i // nsp for i in range(nsp + 1)]
                        for si in range(nsp):
                            a, b = bnds[si], bnds[si + 1]
                            if b == a:
                                continue
                            nc.gpsimd.dma_gather(
                                out_ap=gt[:, a * r2:b * r2]
                                    .rearrange("p (c e) -> p c e", e=r2),
                                in_ap=tbl[:, :],
                                idxs_ap=it[:, (pos + a * P) // 16:
                                           (pos + b * P) // 16],
                                num_idxs=(b - a) * P,
                                num_idxs_reg=(b - a) * P,
                                elem_size=r2,
                                single_packet=False,
                                queue_num=qi[0] % 4,
                            )
                            qi[0] += 1
                        pos += dtot * P
                        gts[h] = gt
                    # ---- pair-merged M: cols [tile0: ex|agg, tile1: ex|agg]
                    npair = len(pair)
                    wmP = npair * wm
                    Js = [int(d0t[t] + d1t[t]) for t in pair]
                    Jmax = max(Js)
                    m = mp.tile([P, Jmax * wmP], F16, tag="m")
                    mv = m[:].rearrange("p (j w) -> p j w", w=wmP)
                    for ti, t in enumerate(pair):
                        J = Js[ti]
                        segs = []
                        for h in (0, 1):
                            dlt = int(dts[h][t])
                            if dlt:
                                segs.append((gts[h], segstart[h][ti], dlt))
                        e = ep.tile([P, J], F32, tag="e")
                        e2 = ep.tile([P, J], F32, tag="e2")
                        co = 0
                        for gt, s0, dlt in segs:
                            gseg = gt[:, s0 * r2:(s0 + dlt) * r2]
                            nc.vector.tensor_scalar(
                                out=e[:, co:co + dlt],
                                in0=gseg.rearrange("p (j r) -> p j r", r=r2)
                                    [:, :, oc:oc + 1]
                                    .rearrange("p j o -> p (j o)"),
                                scalar1=ad[:, t:t + 1], scalar2=None,
                                op0=ALU.add)
                            co += dlt
                        nc.scalar.activation(e2[:], e[:], AF.Lrelu,
                                             alpha=NEG_SLOPE)
                        nc.scalar.activation(
                            mv[:, 0:J, ti * wm:ti * wm + 1]
                                .rearrange("p j o -> p (j o)"),
                            e2[:], AF.Exp)
                        co = 0
                        for gt, s0, dlt in segs:
                            gseg = gt[:, s0 * r2:(s0 + dlt) * r2]
                            nc.vector.tensor_tensor(
                                out=mv[:, co:co + dlt,
                                       ti * wm + 1:ti * wm + wm],
                                in0=gseg.rearrange("p (j r) -> p j r", r=r2)
                                    [:, :, 0:oc],
                                in1=mv[:, co:co + dlt, ti * wm:ti * wm + 1]
                                    .rearrange("p j o -> p (j o)")
                                    .to_broadcast([P, dlt, oc]),
                                op=ALU.mult)
                            co += dlt
                        if J < Jmax:
                            nc.vector.memset(
                                mv[:, J:Jmax, ti * wm:(ti + 1) * wm], 0.0)
                    pt = ps.tile([P, wmP], F32, tag="acc")
                    for j in range(Jmax):
                        nc.tensor.matmul(pt[:], lhsT=id16t[:],
                                         rhs=m[:, j * wmP:(j + 1) * wmP],
                                         start=(j == 0), stop=(j == Jmax - 1))
                    # ---- batched finalize: divide, +b2, log_softmax ----
                    ptv = pt[:].rearrange("p (t w) -> p t w", w=wm)
                    den = smp.tile([P, npair], F32, tag="den")
                    nc.vector.tensor_scalar(
                        out=den[:],
                        in0=ptv[:, :, 0:1].rearrange("p t o -> p (t o)"),
                        scalar1=1e-30, scalar2=None, op0=ALU.max)
                    rec = smp.tile([P, npair], F32, tag="rec")
                    nc.vector.reciprocal(rec[:], den[:])
                    o2b = fin.tile([P, npair * oc], F32, tag="o2b")
                    o2bv = o2b[:].rearrange("p (t c) -> p t c", c=oc)
                    nc.vector.tensor_tensor(
                        out=o2bv, in0=ptv[:, :, 1:wm],
                        in1=rec[:].to_broadcast([P, npair, oc]),
                        op=ALU.mult)
                    nc.vector.tensor_tensor(
                        out=o2bv, in0=o2bv,
                        in1=b2t[:].rearrange("p (o c) -> p o c", o=1)
                            .to_broadcast([P, npair, oc]),
                        op=ALU.add)
                    mx = smp.tile([P, npair], F32, tag="mx")
                    nc.vector.tensor_reduce(out=mx[:], in_=o2bv,
                                            axis=mybir.AxisListType.X,
                                            op=ALU.max)
                    xs = fin.tile([P, npair * oc], F32, tag="xs")
                    xsv = xs[:].rearrange("p (t c) -> p t c", c=oc)
                    nc.vector.tensor_tensor(
                        out=xsv, in0=o2bv,
                        in1=mx[:].to_broadcast([P, npair, oc]),
                        op=ALU.subtract)
                    ex = fin.tile([P, npair * oc], F32, tag="ex")
                    nc.scalar.activation(ex[:], xs[:], AF.Exp)
                    se = smp.tile([P, npair], F32, tag="se")
                    nc.vector.tensor_reduce(
                        out=se[:], in_=ex[:].rearrange("p (t c) -> p t c", c=oc),
                        axis=mybir.AxisListType.X, op=ALU.add)
                    ls = smp.tile([P, npair], F32, tag="ls")
                    nc.scalar.activation(ls[:], se[:], AF.Ln)
                    fo = fin.tile([P, npair * oc], F32, tag="fo")
                    nc.vector.tensor_tensor(
                        out=fo[:].rearrange("p (t c) -> p t c", c=oc),
                        in0=xsv, in1=ls[:].to_broadcast([P, npair, oc]),
                        op=ALU.subtract)
                    t0 = pair[0]
                    nc.sync.dma_start(
                        out=outr[:, t0 * oc:(t0 + npair) * oc], in_=fo[:])
    nc.compile()
    return nc


# ----------------------------------------------------------------------------
# full pipeline
# ----------------------------------------------------------------------------
def run_gat(cfg, inputs, timing=False):
    x = np.asarray(inputs["x"], dtype=np.float32)
    edge_index = np.asarray(inputs["edge_index"])
    W1e, Rinv, W2e = make_consts(
        cfg, np.asarray(inputs["W1"], np.float64),
        np.asarray(inputs["a1_src"], np.float64),
        np.asarray(inputs["a1_dst"], np.float64),
        np.asarray(inputs["W2"], np.float64),
        np.asarray(inputs["a2_src"], np.float64),
        np.asarray(inputs["a2_dst"], np.float64))
    b1 = np.asarray(inputs["b1"], np.float32)
    b2 = np.asarray(inputs["b2"], np.float32)
    pre = preprocess(cfg, edge_index)
    C = cfg.ncores

    # ---- launch A ----
    ncA = build_launchA(cfg)
    rA = SpmdRunner(ncA, C)
    mapsA = []
    for c in range(C):
        g = pre["gids"][c]
        xp = np.zeros((cfg.npad, cfg.in_c), np.float32)
        valid = g >= 0
        xp[np.flatnonzero(valid)] = x[g[valid]]
        mapsA.append({"xT": np.ascontiguousarray(xp.T), "W1e": W1e})
    outsA = rA.results(rA.run(rA.put_inputs(mapsA)))

    # assemble h~ table (fp16) + alpha_d (pi-order per core)
    tblg = np.zeros((cfg.N, cfg.c1), np.float16)
    adsws = []
    for c in range(C):
        g = pre["gids"][c]
        valid = g >= 0
        hr = outsA[c]["hrows"].reshape(P, cfg.ntiles, cfg.c1) \
            .transpose(1, 0, 2).reshape(cfg.npad, cfg.c1)
        tblg[g[valid]] = hr[np.flatnonzero(valid)]
        adsws.append(outsA[c]["adrows"])  # already [P, ntiles*H] packed
    s0, s1 = pre["sloc0"], pre["sloc1"]
    nh = pre["tabrows"]
    tb0 = np.zeros((nh, cfg.c1), np.float16)
    tb1 = np.zeros((nh, cfg.c1), np.float16)
    w0 = s0 >= 0
    tb0[s0[w0]] = tblg[w0]
    w1 = s1 >= 0
    tb1[s1[w1]] = tblg[w1]
    for tb in (tb0, tb1):
        tb[pre["dummy"], 0:cfg.c1:cfg.hid] = DUMMY_ALPHA
    ident = np.eye(P, dtype=np.float32)
    id16 = np.eye(P, dtype=np.float16)

    # ---- launch B ----
    ncB = build_launchB(cfg, pre["d0t"], pre["d1t"], pre["stot16"], nh)
    rB = SpmdRunner(ncB, C)
    mapsB = [{"tb0": tb0, "tb1": tb1, "idxs": pre["idxws"][c],
              "adsw": adsws[c], "ident": ident, "id16": id16, "rinv": Rinv,
              "w2e": W2e, "b1c": b1.reshape(-1, 1)} for c in range(C)]
    outsB = rB.results(rB.run(rB.put_inputs(mapsB)))

    # assemble h2~ table + alpha2_d
    tbl2g = np.zeros((cfg.N, cfg.row2), np.float32)
    ad2sws = []
    for c in range(C):
        g = pre["gids"][c]
        valid = g >= 0
        h2r = outsB[c]["h2rows"].reshape(P, cfg.ntiles, cfg.row2) \
            .transpose(1, 0, 2).reshape(cfg.npad, cfg.row2)
        row = np.zeros((cfg.npad, cfg.row2), np.float32)
        row[:, 0:cfg.out_c + 1] = h2r[:, 0:cfg.out_c + 1]
        tbl2g[g[valid]] = row[np.flatnonzero(valid)]
        ad2 = h2r[:, cfg.out_c + 1]  # [npad] pi-order
        ad2sws.append(np.ascontiguousarray(
            ad2.reshape(cfg.ntiles, P).T))
    tb20 = np.zeros((nh, cfg.row2), np.float32)
    tb21 = np.zeros((nh, cfg.row2), np.float32)
    tb20[s0[w0]] = tbl2g[w0]
    tb21[s1[w1]] = tbl2g[w1]
    for tb in (tb20, tb21):
        tb[pre["dummy"], cfg.out_c] = DUMMY_ALPHA

    # ---- launch C ----
    ncC = build_launchC(cfg, pre["d0t"], pre["d1t"], pre["stot16"], nh)
    rC = SpmdRunner(ncC, C)
    b2bc = np.tile(b2.reshape(1, -1), (P, 1)).astype(np.float32)
    mapsC = [{"tb0": tb20, "tb1": tb21, "idxs": pre["idxws"][c],
              "adsw": ad2sws[c], "id16": id16, "b2c": b2bc}
             for c in range(C)]
    outsC = rC.results(rC.run(rC.put_inputs(mapsC)))

    out = np.zeros((cfg.N, cfg.out_c), np.float32)
    for c in range(C):
        g = pre["gids"][c]
        valid = g >= 0
        our = outsC[c]["outr"].reshape(P, cfg.ntiles, cfg.out_c) \
            .transpose(1, 0, 2).reshape(cfg.npad, cfg.out_c)
        out[g[valid]] = our[np.flatnonzero(valid)]
    return out


def kernel(**inputs) -> np.ndarray:
    return run_gat(CFG, inputs)
